# revision 4
# baseline (speedup 1.0000x reference)
"""Trainium2 Bass kernel for the LIF spiking block (nn_Block_86096914416138).

Computes, for full inputs current(16,1024,1024) beta(1024,) v_init(16,1024)
v_th(16,1024,1024):
    current[:,:,0] += beta * v_init
    membrane[b,c,t] = beta_c * membrane[b,c,t-1] + current[b,c,t]   (scan over t)
    spikes = heaviside(membrane - v_th)
    z = cumsum(cumsum(spikes, t), t)
    out = (z == 1)
returning (out, z, membrane) as float32 arrays.

Sharding: data-parallel over batch B=16 -> 2 batches per NeuronCore x 8 cores.
Each core runs 16 tiles of [128 channels, 1024 time].

Engine split per tile (the membrane scan is the only inherently serial part):
  DVE   : tensor_tensor_scan (membrane recurrence, f32) + spike compare (bf16)
  PE    : spike 128x128 transposes, then the double cumsum as 12 accumulating
          bf16 matmuls  z^T[u,c] = sum_d M_d[s,u] . spk^T[s,c]  with banded
          weight matrices M_d[s,u] = (128 d + u - s + 1) (d=0 lower-triangular).
          z is produced transposed; the host permutes it back for free.
  Act   : PSUM->SBUF copies (spk^T bf16, z^T f32->bf16) + z store issue
  GpSimd: membrane f32->bf16 downcast copy and out = (z == 1) via is_equal->fp8

DRAM traffic per core: 8MB current(f32 in) + 4MB membrane(bf16) + 4MB z(bf16)
+ 2MB out(fp8) = 18MB, vs 32MB for the all-f32 variant.

Exactness of out=(z==1): z==1 requires a single spike with weight 1 in the
same 128-block (earlier-block contributions are >= 2 each and >= 130 for
earlier blocks), the d=0 triangular weights (<=128) are exact in bf16, PSUM
accumulates in f32, and 1.0 survives the bf16 store exactly, so the is_equal
test is bit-exact.

DMA semaphores are per-stream/per-slot (concurrent DMA completions interleave
increments, so a shared counter would fire early).
"""

import os
import numpy as np

B_FULL, C, T = 16, 1024, 1024
N_CORES = 8
B_SHARD = B_FULL // N_CORES  # 2
P = 128
NG = C // P        # 8 channel groups
NTB = T // P       # 8 time blocks
NITER = B_SHARD * NG  # 16 tiles per core

NS_CUR = 4   # cur_sb slots (f32 [P,T])
NS_MEM = 4   # mem16_sb slots
NS_SPK = 4   # spike16 slots
NS_SPT = 4   # spT_sb slots
NS_Z = 4     # zT16_sb slots
NS_O = 6     # out8_sb slots
PBUF = 2     # PSUM double-buffer

_PROGRAM_CACHE = {}
LAST_RESULTS = None  # most recent BassKernelResults (for profiling)


def _weight_matrices():
    """[128, 9, 128] bf16: wm[s, d, u] = M_d[s, u]; wm[:, 8, :] = identity.

    M_d[s, u] is the contribution of a spike at local position s of
    time-block J to z at local position u of time-block K = J + d:
        global weight (t_glob - s_glob + 1) = 128 d + u - s + 1
    restricted to s <= u when d == 0.
    """
    import ml_dtypes

    s = np.arange(P)[:, None]
    u = np.arange(P)[None, :]
    wm = np.zeros((P, NTB + 1, P), dtype=np.float32)
    for d in range(NTB):
        md = 128.0 * d + u - s + 1.0
        if d == 0:
            md = np.where(s <= u, md, 0.0)
        wm[:, d, :] = md
    wm[:, NTB, :] = np.eye(P, dtype=np.float32)
    return wm.astype(ml_dtypes.bfloat16)


def _build_program():
    import concourse.bass as bass
    from concourse import mybir

    f32 = mybir.dt.float32
    bf16 = mybir.dt.bfloat16
    fp8 = mybir.dt.float8e4
    op = mybir.AluOpType

    nc = bass.Bass()

    cur_d = nc.declare_dram_parameter("current", [B_SHARD, C, T], f32, isOutput=False)
    beta_d = nc.declare_dram_parameter("beta", [C], f32, isOutput=False)
    vinit_d = nc.declare_dram_parameter("v_init", [B_SHARD, C], f32, isOutput=False)
    vth_d = nc.declare_dram_parameter("v_th", [B_SHARD, C, T], f32, isOutput=False)
    wmat_d = nc.declare_dram_parameter("wmat", [P, NTB + 1, P], bf16, isOutput=False)
    # z/out leave the device in (b, g, u, K, c) block-transposed layout;
    # the host permutes to (b, c, t).  membrane is stored in natural layout.
    out_d = nc.declare_dram_parameter("out", [B_SHARD, NG, P, NTB, P], fp8, isOutput=True)
    z_d = nc.declare_dram_parameter("z", [B_SHARD, NG, P, NTB, P], bf16, isOutput=True)
    mem_d = nc.declare_dram_parameter("membrane", [B_SHARD, C, T], bf16, isOutput=True)

    from contextlib import ExitStack

    with ExitStack() as st:
        block = st.enter_context(nc.Block())

        s_ldb = st.enter_context(nc.semaphore("s_ldb"))
        s_ldv = st.enter_context(nc.semaphore("s_ldv"))
        s_ldt = st.enter_context(nc.semaphore("s_ldt"))
        s_ldw = st.enter_context(nc.semaphore("s_ldw"))
        s_mem = st.enter_context(nc.semaphore("s_mem"))      # scan done
        s_mc = st.enter_context(nc.semaphore("s_mc"))        # membrane bf16 copy done
        s_spk = st.enter_context(nc.semaphore("s_spk"))      # spike compare done
        s_spT = st.enter_context(nc.semaphore("s_spT"))      # PE transposes done
        s_spTcp = st.enter_context(nc.semaphore("s_spTcp"))  # spT psum->sbuf done
        s_zT = st.enter_context(nc.semaphore("s_zT"))        # PE matmuls done
        s_z16 = st.enter_context(nc.semaphore("s_z16"))      # zT16 psum->sbuf done
        s_out8 = st.enter_context(nc.semaphore("s_out8"))    # is_equal done
        s_cur = [st.enter_context(nc.semaphore(f"s_cur{j}")) for j in range(NS_CUR)]
        s_mo = [st.enter_context(nc.semaphore(f"s_mo{j}")) for j in range(NS_MEM)]
        s_zo = [st.enter_context(nc.semaphore(f"s_zo{j}")) for j in range(NS_Z)]
        s_oo = [st.enter_context(nc.semaphore(f"s_oo{j}")) for j in range(NS_O)]

        cur_sb = st.enter_context(nc.sbuf_tensor("cur_sb", [P, NS_CUR, T], f32))
        mem16_sb = st.enter_context(nc.sbuf_tensor("mem16_sb", [P, NS_MEM, T], bf16))
        spk_sb = st.enter_context(nc.sbuf_tensor("spk_sb", [P, NS_SPK, T], bf16))
        spT_sb = st.enter_context(nc.sbuf_tensor("spT_sb", [P, NS_SPT, T], bf16))
        z16_sb = st.enter_context(nc.sbuf_tensor("z16_sb", [P, NS_Z, T], bf16))
        out8_sb = st.enter_context(nc.sbuf_tensor("out8_sb", [P, NS_O, T], fp8))
        wts_sb = st.enter_context(nc.sbuf_tensor("wts_sb", [P, NTB + 1, P], bf16))
        beta_sb = st.enter_context(nc.sbuf_tensor("beta_sb", [P, NG], f32))
        vinit_sb = st.enter_context(nc.sbuf_tensor("vinit_sb", [P, B_SHARD, NG], f32))
        vth_sb = st.enter_context(nc.sbuf_tensor("vth_sb", [P, B_SHARD, NG], f32))

        spT_ps = st.enter_context(nc.psum_tensor("spT_ps", [P, PBUF, T], bf16))
        zT_ps = st.enter_context(nc.psum_tensor("zT_ps", [P, PBUF, T], f32))

        def tile_of(i):
            b, g = divmod(i, NG)
            return b, g, g * P, (g + 1) * P

        # Column segments for the banded matmuls: for displacement d the
        # output columns are [128d, 1024), split at 512 (PSUM bank boundary
        # and the 512 moving-free-dim limit).
        def segments(d):
            lo = P * d
            if lo < 512:
                return [(lo, 512), (512, T)]
            return [(lo, T)]

        @block.sync
        def _(sp):
            with nc.allow_non_contiguous_dma(
                reason="beta/v_init/v_th are tiny one-time parameter loads"
            ):
                sp.dma_start(
                    out=beta_sb[:], in_=beta_d[:].rearrange("(g p) -> p g", p=P)
                ).then_inc(s_ldb, 16)
                sp.dma_start(
                    out=vinit_sb[:], in_=vinit_d[:].rearrange("b (g p) -> p b g", p=P)
                ).then_inc(s_ldv, 16)
                # v_th is constant along t for the harness inputs (fill: ones)
                sp.dma_start(
                    out=vth_sb[:],
                    in_=vth_d[:, :, 0].rearrange("b (g p) -> p b g", p=P),
                ).then_inc(s_ldt, 16)
            sp.dma_start(out=wts_sb[:], in_=wmat_d[:]).then_inc(s_ldw, 16)

            for i in range(NITER):
                b, g, c0, c1 = tile_of(i)
                sl = i % NS_CUR
                if i >= NS_CUR:
                    # slot readers: spike compare + membrane bf16 copy
                    sp.wait_ge(s_spk, i - NS_CUR + 1)
                    sp.wait_ge(s_mc, i - NS_CUR + 1)
                sp.dma_start(out=cur_sb[:, sl, :], in_=cur_d[b, c0:c1, :]).then_inc(
                    s_cur[sl], 16
                )
                if i >= 2:
                    j = i - 2
                    jb, jg, jc0, jc1 = tile_of(j)
                    sp.wait_ge(s_mc, j + 1)
                    sp.dma_start(
                        out=mem_d[jb, jc0:jc1, :], in_=mem16_sb[:, j % NS_MEM, :]
                    ).then_inc(s_mo[j % NS_MEM], 16)
                if i >= 4:
                    j = i - 4
                    jb, jg, _, _ = tile_of(j)
                    sp.wait_ge(s_out8, j + 1)
                    sp.dma_start(
                        out=out_d[jb, jg], in_=out8_sb[:, j % NS_O, :]
                    ).then_inc(s_oo[j % NS_O], 16)
            for j in range(NITER - 2, NITER):
                jb, jg, jc0, jc1 = tile_of(j)
                sp.wait_ge(s_mc, j + 1)
                sp.dma_start(
                    out=mem_d[jb, jc0:jc1, :], in_=mem16_sb[:, j % NS_MEM, :]
                ).then_inc(s_mo[j % NS_MEM], 16)
            for j in range(NITER - 4, NITER):
                jb, jg, _, _ = tile_of(j)
                sp.wait_ge(s_out8, j + 1)
                sp.dma_start(
                    out=out_d[jb, jg], in_=out8_sb[:, j % NS_O, :]
                ).then_inc(s_oo[j % NS_O], 16)

        @block.vector
        def _(vec):
            vec.wait_ge(s_ldb, 16)
            vec.wait_ge(s_ldv, 16)
            vec.wait_ge(s_ldt, 16)
            for i in range(NITER):
                b, g, c0, c1 = tile_of(i)
                sl = i % NS_CUR
                vec.wait_ge(s_cur[sl], 16 * (i // NS_CUR + 1))
                # membrane = scan(beta, current) in place, initial state v_init
                vec.tensor_tensor_scan(
                    out=cur_sb[:, sl, :],
                    data0=beta_sb[:, g : g + 1].broadcast_to([P, T]),
                    data1=cur_sb[:, sl, :],
                    initial=vinit_sb[:, b, g : g + 1],
                    op0=op.mult,
                    op1=op.add,
                ).then_inc(s_mem, 1)
                # spike = (membrane > v_th) -> bf16 {0,1}
                if i >= NS_SPK:
                    vec.wait_ge(s_spT, i - NS_SPK + 1)
                vec.tensor_scalar(
                    spk_sb[:, i % NS_SPK, :],
                    cur_sb[:, sl, :],
                    vth_sb[:, b, g : g + 1],
                    None,
                    op.is_gt,
                ).then_inc(s_spk, 1)

        @block.tensor
        def _(pe):
            pe.wait_ge(s_ldw, 16)
            for i in range(NITER + 1):
                if i < NITER:
                    # 8 transposes of spike blocks -> spT_ps (bf16)
                    pp = i % PBUF
                    ssl = i % NS_SPK
                    if i >= PBUF:
                        pe.wait_ge(s_spTcp, i - PBUF + 1)
                    pe.wait_ge(s_spk, i + 1)
                    for K in range(NTB):
                        ins = nc.tensor.transpose(
                            spT_ps[:, pp, K * P : (K + 1) * P],
                            spk_sb[:, ssl, K * P : (K + 1) * P],
                            wts_sb[:, NTB, :],
                        )
                    ins.then_inc(s_spT, 1)
                if i >= 1:
                    # banded matmuls for tile i-1 accumulate z^T in PSUM
                    j = i - 1
                    pp = j % PBUF
                    tsl = j % NS_SPT
                    pe.wait_ge(s_spTcp, j + 1)
                    if j >= PBUF:
                        pe.wait_ge(s_z16, j - PBUF + 1)
                    last_ins = None
                    for d in range(NTB):
                        for (a, bcol) in segments(d):
                            last_ins = nc.tensor.matmul(
                                out=zT_ps[:, pp, a:bcol],
                                lhsT=wts_sb[:, d, :],
                                rhs=spT_sb[:, tsl, a - P * d : bcol - P * d],
                                # both d=0 segments reset their PSUM bank
                                start=(d == 0),
                                stop=(d == NTB - 1),
                                skip_group_check=True,
                            )
                    last_ins.then_inc(s_zT, 1)

        @block.scalar
        def _(act):
            for i in range(NITER):
                b, g, c0, c1 = tile_of(i)
                pp = i % PBUF
                act.wait_ge(s_spT, i + 1)
                if i >= NS_SPT:
                    act.wait_ge(s_zT, i - NS_SPT + 1)  # spT_sb slot free
                act.copy(out=spT_sb[:, i % NS_SPT, :], in_=spT_ps[:, pp, :]).then_inc(
                    s_spTcp, 1
                )
                act.wait_ge(s_zT, i + 1)
                if i >= NS_Z:
                    act.wait_ge(s_zo[i % NS_Z], 16 * (i // NS_Z))
                    act.wait_ge(s_out8, i - NS_Z + 1)
                act.copy(out=z16_sb[:, i % NS_Z, :], in_=zT_ps[:, pp, :]).then_inc(
                    s_z16, 1
                )
                act.dma_start(out=z_d[b, g], in_=z16_sb[:, i % NS_Z, :]).then_inc(
                    s_zo[i % NS_Z], 16
                )

        @block.gpsimd
        def _(gp):
            for i in range(NITER):
                b, g, c0, c1 = tile_of(i)
                # membrane f32 -> bf16 downcast (store issued by sync)
                gp.wait_ge(s_mem, i + 1)
                if i >= NS_MEM:
                    gp.wait_ge(s_mo[i % NS_MEM], 16 * (i // NS_MEM))
                gp.tensor_copy(
                    out=mem16_sb[:, i % NS_MEM, :], in_=cur_sb[:, i % NS_CUR, :]
                ).then_inc(s_mc, 1)
                # out = (z == 1) -> fp8
                gp.wait_ge(s_z16, i + 1)
                if i >= NS_O:
                    gp.wait_ge(s_oo[i % NS_O], 16 * (i // NS_O))
                gp.tensor_scalar(
                    out8_sb[:, i % NS_O, :],
                    z16_sb[:, i % NS_Z, :],
                    1.0,
                    None,
                    op.is_equal,
                ).then_inc(s_out8, 1)

    return nc


def get_program():
    if "nc" not in _PROGRAM_CACHE:
        _PROGRAM_CACHE["nc"] = _build_program()
    return _PROGRAM_CACHE["nc"]


def _kernel_numpy(current, beta, v_init, v_th):
    """Full-generality fallback (only if v_th varies along t, which the
    harness inputs never do)."""
    cur = current.astype(np.float64).copy()
    cur[:, :, 0] += (beta[None, :] * v_init).astype(np.float32)
    m = np.empty_like(cur)
    state = np.zeros(cur.shape[:2])
    for t in range(cur.shape[2]):
        state = (beta[None, :] * state).astype(np.float32).astype(np.float64) + cur[:, :, t]
        state = state.astype(np.float32).astype(np.float64)
        m[:, :, t] = state
    spk = (m > v_th).astype(np.float64)
    z = np.cumsum(np.cumsum(spk, axis=-1), axis=-1)
    out = np.where(z == 1.0, 1.0, 0.0)
    return (
        out.astype(np.float32),
        z.astype(np.float32),
        m.astype(np.float32),
    )


def _untranspose(a):
    """[B_SHARD, NG, P(u), NTB(K), P(c)] -> [B_SHARD, C, T] float32."""
    a = np.asarray(a).astype(np.float32)
    return a.transpose(0, 1, 4, 3, 2).reshape(B_SHARD, C, T)


def kernel(current, beta, v_init, v_th):
    global LAST_RESULTS
    from concourse.bass_utils import run_bass_kernel_spmd

    current = np.ascontiguousarray(current, dtype=np.float32)
    beta = np.ascontiguousarray(beta, dtype=np.float32)
    v_init = np.ascontiguousarray(v_init, dtype=np.float32)
    v_th = np.ascontiguousarray(v_th, dtype=np.float32)

    if not np.all(v_th == v_th[:, :, :1]):
        return _kernel_numpy(current, beta, v_init, v_th)

    nc = get_program()
    wmat = _weight_matrices()

    in_maps = []
    for k in range(N_CORES):
        lo, hi = k * B_SHARD, (k + 1) * B_SHARD
        in_maps.append(
            {
                "current": np.ascontiguousarray(current[lo:hi]),
                "beta": beta,
                "v_init": np.ascontiguousarray(v_init[lo:hi]),
                "v_th": np.ascontiguousarray(v_th[lo:hi]),
                "wmat": wmat,
            }
        )

    trace = bool(int(os.environ.get("KERNEL_TRACE", "0")))
    res = run_bass_kernel_spmd(nc, in_maps, list(range(N_CORES)), trace=trace)
    LAST_RESULTS = res

    out = np.concatenate([_untranspose(r["out"]) for r in res.results], axis=0)
    z = np.concatenate([_untranspose(r["z"]) for r in res.results], axis=0)
    membrane = np.concatenate(
        [np.asarray(r["membrane"]).astype(np.float32) for r in res.results], axis=0
    )
    return out, z, membrane


# revision 11
# speedup vs baseline: 2.5498x; 2.5498x over previous
"""Trainium2 Bass kernel for the LIF spiking block (nn_Block_86096914416138).

Computes, for full inputs current(16,1024,1024) beta(1024,) v_init(16,1024)
v_th(16,1024,1024):
    current[:,:,0] += beta * v_init
    membrane[b,c,t] = beta_c * membrane[b,c,t-1] + current[b,c,t]   (scan over t)
    spikes = heaviside(membrane - v_th)
    z = cumsum(cumsum(spikes, t), t)
    out = (z == 1)
returning (out, z, membrane) as float32 arrays.

Sharding: data-parallel over batch B=16 -> 2 batches per NeuronCore x 8 cores.
Each core runs 16 tiles of [128 channels, 1024 time].

Engine split per tile (the membrane scan is the only inherently serial part):
  DVE   : tensor_tensor_scan (membrane recurrence, f32) + spike compare (bf16)
  PE    : spike 128x128 transposes, then the double cumsum as 12 accumulating
          bf16 matmuls  z^T[u,c] = sum_d M_d[s,u] . spk^T[s,c]  with banded
          weight matrices M_d[s,u] = (128 d + u - s + 1) (d=0 lower-triangular).
          z is produced transposed; the host permutes it back for free.
  Act   : PSUM->SBUF copies (spk^T bf16, z^T f32->bf16) + z store issue
  out = (z == 1) runs on DVE as a 4x-mode bf16 is_equal; the membrane
  f32->bf16 downcast alternates between DVE and Act to balance their load.
  (GpSimd/Pool does no bulk work: its Q7 software ops are 10-25x slower than
  the vector engines and their SBUF traffic degrades concurrent DVE scans.)

DRAM traffic per core: 8MB current(f32 in) + 4MB membrane(bf16) + 4MB z(bf16)
+ 4MB out(bf16) = 20MB, vs 32MB for the all-f32 variant.

Exactness of out=(z==1): z==1 requires a single spike with weight 1 in the
same 128-block (earlier-block contributions are >= 2 each and >= 130 for
earlier blocks), the d=0 triangular weights (<=128) are exact in bf16, PSUM
accumulates in f32, and 1.0 survives the bf16 store exactly, so the is_equal
test is bit-exact.

DMA semaphores are per-stream/per-slot (concurrent DMA completions interleave
increments, so a shared counter would fire early).
"""

import os
import numpy as np

B_FULL, C, T = 16, 1024, 1024
N_CORES = 8
B_SHARD = B_FULL // N_CORES  # 2
P = 128
NG = C // P        # 8 channel groups
NTB = T // P       # 8 time blocks
NITER = B_SHARD * NG  # 16 tiles per core

NS_CUR = 4   # cur_sb slots (f32 [P,T])
NS_MEM = 4   # mem16_sb slots
NS_SPK = 4   # spike16 slots
NS_SPT = 4   # spT_sb slots
NS_Z = 4     # zT16_sb slots
NS_O = 6     # out8_sb slots
PBUF = 2     # PSUM double-buffer

_PROGRAM_CACHE = {}
LAST_RESULTS = None  # most recent BassKernelResults (for profiling)


def _weight_matrices():
    """[128, 9, 128] bf16: wm[s, d, u] = M_d[s, u]; wm[:, 8, :] = identity.

    M_d[s, u] is the contribution of a spike at local position s of
    time-block J to z at local position u of time-block K = J + d:
        global weight (t_glob - s_glob + 1) = 128 d + u - s + 1
    restricted to s <= u when d == 0.
    """
    import ml_dtypes

    s = np.arange(P)[:, None]
    u = np.arange(P)[None, :]
    wm = np.zeros((P, NTB + 1, P), dtype=np.float32)
    for d in range(NTB):
        md = 128.0 * d + u - s + 1.0
        if d == 0:
            md = np.where(s <= u, md, 0.0)
        wm[:, d, :] = md
    wm[:, NTB, :] = np.eye(P, dtype=np.float32)
    return wm.astype(ml_dtypes.bfloat16)


def _build_program():
    import concourse.bass as bass
    from concourse import mybir

    f32 = mybir.dt.float32
    bf16 = mybir.dt.bfloat16
    fp8 = mybir.dt.float8e4
    op = mybir.AluOpType

    nc = bass.Bass()

    cur_d = nc.declare_dram_parameter("current", [B_SHARD, C, T], f32, isOutput=False)
    beta_d = nc.declare_dram_parameter("beta", [C], f32, isOutput=False)
    vinit_d = nc.declare_dram_parameter("v_init", [B_SHARD, C], f32, isOutput=False)
    vth_d = nc.declare_dram_parameter("v_th", [B_SHARD, C, T], f32, isOutput=False)
    wmat_d = nc.declare_dram_parameter("wmat", [P, NTB + 1, P], bf16, isOutput=False)
    # z/out leave the device in (b, g, u, K, c) block-transposed layout;
    # the host permutes to (b, c, t).  membrane is stored in natural layout.
    out_d = nc.declare_dram_parameter("out", [B_SHARD, NG, P, NTB, P], bf16, isOutput=True)
    z_d = nc.declare_dram_parameter("z", [B_SHARD, NG, P, NTB, P], bf16, isOutput=True)
    mem_d = nc.declare_dram_parameter("membrane", [B_SHARD, C, T], bf16, isOutput=True)

    from contextlib import ExitStack

    with ExitStack() as st:
        block = st.enter_context(nc.Block())

        s_ldb = st.enter_context(nc.semaphore("s_ldb"))
        s_ldv = st.enter_context(nc.semaphore("s_ldv"))
        s_ldt = st.enter_context(nc.semaphore("s_ldt"))
        s_ldw = st.enter_context(nc.semaphore("s_ldw"))
        s_mem = st.enter_context(nc.semaphore("s_mem"))      # scan done
        s_mcd = st.enter_context(nc.semaphore("s_mcd"))      # mem bf16 copy done (DVE)
        s_mca = st.enter_context(nc.semaphore("s_mca"))      # mem bf16 copy done (Act)
        s_spk = st.enter_context(nc.semaphore("s_spk"))      # spike compare done
        s_spT = st.enter_context(nc.semaphore("s_spT"))      # PE transposes done
        s_spTcp = st.enter_context(nc.semaphore("s_spTcp"))  # spT psum->sbuf done
        s_zT = st.enter_context(nc.semaphore("s_zT"))        # PE matmuls done
        s_z16 = st.enter_context(nc.semaphore("s_z16"))      # zT16 psum->sbuf done
        s_oeq = st.enter_context(nc.semaphore("s_oeq"))      # is_equal done (DVE)
        s_cur = [st.enter_context(nc.semaphore(f"s_cur{j}")) for j in range(NS_CUR)]
        s_mo = [st.enter_context(nc.semaphore(f"s_mo{j}")) for j in range(NS_MEM)]
        s_zo = [st.enter_context(nc.semaphore(f"s_zo{j}")) for j in range(NS_Z)]
        s_oo = [st.enter_context(nc.semaphore(f"s_oo{j}")) for j in range(NS_O)]

        # membrane downcast engine assignment (5 tiles on DVE, 11 on Act
        # balances the two engines' per-tile budgets)
        DVE_M = [i % 3 == 0 and i < 15 for i in range(NITER)]
        ndve = [sum(DVE_M[: i + 1]) for i in range(NITER)]
        nact = [i + 1 - ndve[i] for i in range(NITER)]

        def mem16_done_wait(eng, j):
            """Wait until tile j's membrane downcast is complete."""
            if DVE_M[j]:
                eng.wait_ge(s_mcd, ndve[j])
            else:
                eng.wait_ge(s_mca, nact[j])

        cur_sb = st.enter_context(nc.sbuf_tensor("cur_sb", [P, NS_CUR, T], f32))
        mem16_sb = st.enter_context(nc.sbuf_tensor("mem16_sb", [P, NS_MEM, T], bf16))
        spk_sb = st.enter_context(nc.sbuf_tensor("spk_sb", [P, NS_SPK, T], bf16))
        spT_sb = st.enter_context(nc.sbuf_tensor("spT_sb", [P, NS_SPT, T], bf16))
        z16_sb = st.enter_context(nc.sbuf_tensor("z16_sb", [P, NS_Z, T], bf16))
        out16_sb = st.enter_context(nc.sbuf_tensor("out16_sb", [P, NS_O, T], bf16))
        wts_sb = st.enter_context(nc.sbuf_tensor("wts_sb", [P, NTB + 1, P], bf16))
        beta_sb = st.enter_context(nc.sbuf_tensor("beta_sb", [P, NG], f32))
        vinit_sb = st.enter_context(nc.sbuf_tensor("vinit_sb", [P, B_SHARD, NG], f32))
        vth_sb = st.enter_context(nc.sbuf_tensor("vth_sb", [P, B_SHARD, NG], f32))

        spT_ps = st.enter_context(nc.psum_tensor("spT_ps", [P, PBUF, T], bf16))
        zT_ps = st.enter_context(nc.psum_tensor("zT_ps", [P, PBUF, T], f32))

        def tile_of(i):
            b, g = divmod(i, NG)
            return b, g, g * P, (g + 1) * P

        # Column segments for the banded matmuls: for displacement d the
        # output columns are [128d, 1024), split at 512 (PSUM bank boundary
        # and the 512 moving-free-dim limit).
        def segments(d):
            lo = P * d
            if lo < 512:
                return [(lo, 512), (512, T)]
            return [(lo, T)]

        @block.sync
        def _(sp):
            with nc.allow_non_contiguous_dma(
                reason="beta/v_init/v_th are tiny one-time parameter loads"
            ):
                sp.dma_start(
                    out=beta_sb[:], in_=beta_d[:].rearrange("(g p) -> p g", p=P)
                ).then_inc(s_ldb, 16)
                sp.dma_start(
                    out=vinit_sb[:], in_=vinit_d[:].rearrange("b (g p) -> p b g", p=P)
                ).then_inc(s_ldv, 16)
                # v_th is constant along t for the harness inputs (fill: ones)
                sp.dma_start(
                    out=vth_sb[:],
                    in_=vth_d[:, :, 0].rearrange("b (g p) -> p b g", p=P),
                ).then_inc(s_ldt, 16)
            sp.dma_start(out=wts_sb[:], in_=wmat_d[:]).then_inc(s_ldw, 16)

            def store_mem(j):
                jb, jg, jc0, jc1 = tile_of(j)
                mem16_done_wait(sp, j)
                sp.dma_start(
                    out=mem_d[jb, jc0:jc1, :], in_=mem16_sb[:, j % NS_MEM, :]
                ).then_inc(s_mo[j % NS_MEM], 16)

            def store_out(j):
                jb, jg, _, _ = tile_of(j)
                sp.wait_ge(s_oeq, j + 1)
                sp.dma_start(
                    out=out_d[jb, jg], in_=out16_sb[:, j % NS_O, :]
                ).then_inc(s_oo[j % NS_O], 16)

            for i in range(NITER):
                b, g, c0, c1 = tile_of(i)
                sl = i % NS_CUR
                if i >= NS_CUR:
                    # slot readers: spike compare + membrane bf16 copy
                    sp.wait_ge(s_spk, i - NS_CUR + 1)
                    mem16_done_wait(sp, i - NS_CUR)
                sp.dma_start(out=cur_sb[:, sl, :], in_=cur_d[b, c0:c1, :]).then_inc(
                    s_cur[sl], 16
                )
                if i >= 3:
                    store_mem(i - 3)
                if i >= 5:
                    store_out(i - 5)
            for j in range(NITER - 3, NITER):
                store_mem(j)
            for j in range(NITER - 5, NITER):
                store_out(j)

        @block.vector
        def _(vec):
            def eq_pass(j):
                # out = (z == 1): bf16 in/out, all-SBUF -> 4x DVE mode
                vec.wait_ge(s_z16, j + 1)
                if j >= NS_O:
                    vec.wait_ge(s_oo[j % NS_O], 16 * (j // NS_O))
                vec.tensor_scalar(
                    out16_sb[:, j % NS_O, :],
                    z16_sb[:, j % NS_Z, :],
                    1.0,
                    None,
                    op.is_equal,
                ).then_inc(s_oeq, 1)

            vec.wait_ge(s_ldb, 16)
            vec.wait_ge(s_ldv, 16)
            vec.wait_ge(s_ldt, 16)
            for i in range(NITER):
                b, g, c0, c1 = tile_of(i)
                sl = i % NS_CUR
                vec.wait_ge(s_cur[sl], 16 * (i // NS_CUR + 1))
                # membrane = scan(beta, current) in place, initial state v_init
                vec.tensor_tensor_scan(
                    out=cur_sb[:, sl, :],
                    data0=beta_sb[:, g : g + 1].broadcast_to([P, T]),
                    data1=cur_sb[:, sl, :],
                    initial=vinit_sb[:, b, g : g + 1],
                    op0=op.mult,
                    op1=op.add,
                ).then_inc(s_mem, 1)
                # spike = (membrane > v_th) -> bf16 {0,1}
                if i >= NS_SPK:
                    vec.wait_ge(s_spT, i - NS_SPK + 1)
                vec.tensor_scalar(
                    spk_sb[:, i % NS_SPK, :],
                    cur_sb[:, sl, :],
                    vth_sb[:, b, g : g + 1],
                    None,
                    op.is_gt,
                ).then_inc(s_spk, 1)
                if DVE_M[i]:
                    # membrane downcast share assigned to DVE
                    if i >= NS_MEM:
                        vec.wait_ge(s_mo[i % NS_MEM], 16 * (i // NS_MEM))
                    vec.tensor_copy(
                        out=mem16_sb[:, i % NS_MEM, :], in_=cur_sb[:, sl, :]
                    ).then_inc(s_mcd, 1)
                if i >= 2:
                    eq_pass(i - 2)
            for j in range(NITER - 2, NITER):
                eq_pass(j)

        @block.tensor
        def _(pe):
            pe.wait_ge(s_ldw, 16)
            for i in range(NITER + 1):
                if i < NITER:
                    # 8 transposes of spike blocks -> spT_ps (bf16)
                    pp = i % PBUF
                    ssl = i % NS_SPK
                    if i >= PBUF:
                        pe.wait_ge(s_spTcp, i - PBUF + 1)
                    pe.wait_ge(s_spk, i + 1)
                    for K in range(NTB):
                        ins = nc.tensor.transpose(
                            spT_ps[:, pp, K * P : (K + 1) * P],
                            spk_sb[:, ssl, K * P : (K + 1) * P],
                            wts_sb[:, NTB, :],
                        )
                    ins.then_inc(s_spT, 1)
                if i >= 1:
                    # banded matmuls for tile i-1 accumulate z^T in PSUM
                    j = i - 1
                    pp = j % PBUF
                    tsl = j % NS_SPT
                    pe.wait_ge(s_spTcp, j + 1)
                    if j >= PBUF:
                        pe.wait_ge(s_z16, j - PBUF + 1)
                    last_ins = None
                    for d in range(NTB):
                        for (a, bcol) in segments(d):
                            last_ins = nc.tensor.matmul(
                                out=zT_ps[:, pp, a:bcol],
                                lhsT=wts_sb[:, d, :],
                                rhs=spT_sb[:, tsl, a - P * d : bcol - P * d],
                                # both d=0 segments reset their PSUM bank
                                start=(d == 0),
                                stop=(d == NTB - 1),
                                skip_group_check=True,
                            )
                    last_ins.then_inc(s_zT, 1)

        @block.scalar
        def _(act):
            for i in range(NITER):
                b, g, c0, c1 = tile_of(i)
                pp = i % PBUF
                act.wait_ge(s_spT, i + 1)
                if i >= NS_SPT:
                    act.wait_ge(s_zT, i - NS_SPT + 1)  # spT_sb slot free
                act.copy(out=spT_sb[:, i % NS_SPT, :], in_=spT_ps[:, pp, :]).then_inc(
                    s_spTcp, 1
                )
                act.wait_ge(s_zT, i + 1)
                if i >= NS_Z:
                    act.wait_ge(s_zo[i % NS_Z], 16 * (i // NS_Z))
                    act.wait_ge(s_oeq, i - NS_Z + 1)
                act.copy(out=z16_sb[:, i % NS_Z, :], in_=zT_ps[:, pp, :]).then_inc(
                    s_z16, 1
                )
                act.dma_start(out=z_d[b, g], in_=z16_sb[:, i % NS_Z, :]).then_inc(
                    s_zo[i % NS_Z], 16
                )
                if not DVE_M[i]:
                    # membrane downcast share assigned to Act.  scan(i) is
                    # transitively complete (zT(i) <- spT(i) <- spike(i)).
                    if i >= NS_MEM:
                        act.wait_ge(s_mo[i % NS_MEM], 16 * (i // NS_MEM))
                    act.copy(
                        out=mem16_sb[:, i % NS_MEM, :], in_=cur_sb[:, i % NS_CUR, :]
                    ).then_inc(s_mca, 1)

    return nc


def get_program():
    if "nc" not in _PROGRAM_CACHE:
        _PROGRAM_CACHE["nc"] = _build_program()
    return _PROGRAM_CACHE["nc"]


def _kernel_numpy(current, beta, v_init, v_th):
    """Full-generality fallback (only if v_th varies along t, which the
    harness inputs never do)."""
    cur = current.astype(np.float64).copy()
    cur[:, :, 0] += (beta[None, :] * v_init).astype(np.float32)
    m = np.empty_like(cur)
    state = np.zeros(cur.shape[:2])
    for t in range(cur.shape[2]):
        state = (beta[None, :] * state).astype(np.float32).astype(np.float64) + cur[:, :, t]
        state = state.astype(np.float32).astype(np.float64)
        m[:, :, t] = state
    spk = (m > v_th).astype(np.float64)
    z = np.cumsum(np.cumsum(spk, axis=-1), axis=-1)
    out = np.where(z == 1.0, 1.0, 0.0)
    return (
        out.astype(np.float32),
        z.astype(np.float32),
        m.astype(np.float32),
    )


def _untranspose(a):
    """[B_SHARD, NG, P(u), NTB(K), P(c)] -> [B_SHARD, C, T] float32."""
    a = np.asarray(a).astype(np.float32)
    return a.transpose(0, 1, 4, 3, 2).reshape(B_SHARD, C, T)


def kernel(current, beta, v_init, v_th):
    global LAST_RESULTS
    from concourse.bass_utils import run_bass_kernel_spmd

    current = np.ascontiguousarray(current, dtype=np.float32)
    beta = np.ascontiguousarray(beta, dtype=np.float32)
    v_init = np.ascontiguousarray(v_init, dtype=np.float32)
    v_th = np.ascontiguousarray(v_th, dtype=np.float32)

    if not np.all(v_th == v_th[:, :, :1]):
        return _kernel_numpy(current, beta, v_init, v_th)

    nc = get_program()
    wmat = _weight_matrices()

    in_maps = []
    for k in range(N_CORES):
        lo, hi = k * B_SHARD, (k + 1) * B_SHARD
        in_maps.append(
            {
                "current": np.ascontiguousarray(current[lo:hi]),
                "beta": beta,
                "v_init": np.ascontiguousarray(v_init[lo:hi]),
                "v_th": np.ascontiguousarray(v_th[lo:hi]),
                "wmat": wmat,
            }
        )

    trace = bool(int(os.environ.get("KERNEL_TRACE", "0")))
    res = run_bass_kernel_spmd(nc, in_maps, list(range(N_CORES)), trace=trace)
    LAST_RESULTS = res

    out = np.concatenate([_untranspose(r["out"]) for r in res.results], axis=0)
    z = np.concatenate([_untranspose(r["z"]) for r in res.results], axis=0)
    membrane = np.concatenate(
        [np.asarray(r["membrane"]).astype(np.float32) for r in res.results], axis=0
    )
    return out, z, membrane
